# revision 12
# baseline (speedup 1.0000x reference)
"""AirGC GNN message-passing kernel for 8 Trainium2 NeuronCores.

Strategy
--------
Nodes are relabeled so core c owns the nodes that become h-features
[20000c, 20000(c+1)) of the BN-MLP head: orig node v=(g,n) with g=graph,
n=node-in-graph maps to core j=n//50, local row (n%50)*128+g.  Each core
owns 6400 destination rows (50 blocks of 128; partition = graph id).

K=10 propagation steps (gamma*2*(1-lam)==1 so y = A_hat z):
  - z~ = dinv * z is kept as an fp16 DRAM gather table of paired rows
    [25600, 1024] (node n at pair row n//2, column half n%2; 400 feats
    padded to 512 so each half is a 1024B 256-multiple).
  - per dst block of 128 nodes, in-edges (incl. self loops) are fetched
    with dma_gather (int16 pair-row indices, elem_step=1024 fp16).
    Chunks of 128 edges are merged into large calls (CAP chunks each,
    split by source parity, parity-interleaved in block order) to
    amortize the ~1us/call SWDGE fixed cost; Q7 descriptor generation
    (~7.3ns/idx) is the critical-path wall.
  - segment-sum via TensorE: psum[128dst,400] += Sel^T[128e,128d] @ G,
    Sel built on VectorE with is_equal(dst_local, iota).
  - AirGC update: d = dinv_dst*psum - x; rn=|d|; s=relu(rn-.5)/rn;
    z' = x + s*d; table row = dinv*z' (fp16).
  - The table is double-buffered in pair-shared ("Shared") HBM; the
    inter-step exchange is an AllGather split into 4 block-quarters so
    the first 3 quarters overlap the tail of the gather burst and only
    the last quarter's latency is exposed.
Head: fused into step K-1's block loop: transposed blocks put
h-features on partitions so BN0 stats are row reductions; (h-mu)*rinv
contracts with host-side bn0_g-folded w1 (fp16) into [128,128]
partials; one 64KB AllReduce; BN1..3 + MLP tail replicated per core
(stats over graphs via ones-matmul).
"""

import os
import sys

for _p in ("/opt/trn_rl_repo", "/root/.axon_site/_ro/trn_rl_repo"):
    if os.path.isdir(_p) and _p not in sys.path:
        sys.path.insert(0, _p)

import numpy as np

from concourse import bacc, bass, mybir, tile
from concourse.bass_utils import run_bass_kernel_spmd
from concourse.library_config import mlp as _mlp_lib

F16 = mybir.dt.float16
F32 = mybir.dt.float32
I16 = mybir.dt.int16
OP = mybir.AluOpType
AF = mybir.ActivationFunctionType

N = 51200          # nodes
F = 400            # features per node
G = 128            # graphs
NPG = 400          # nodes per graph
CORES = 8
NPC = N // CORES   # 6400 nodes per core
NSLOT = NPG // CORES  # 50 node-slots per core == blocks per core
K = int(os.environ.get('GNN_K', '10'))
CAP = int(os.environ.get('GNN_CAP', '5'))  # >5 chunks/call hangs SWDGE
LAM = 0.5
BN_EPS = 1e-5
DP = 512           # padded feature columns (fp16, 1024B)
PR = N // 2        # 25600 pair rows in gather table
H1, H2, NCLS = 128, 64, 10
# AllGather split boundaries (block index); table is quarter-major so each
# AG quarter writes one contiguous slice:
# row = QOFF[q] + core*QROWS[q] + local-pair-within-quarter
# NOTE: splitting the AG (e.g. '13,26,38,50') deadlocks on device —
# offset-output collectives are not safe here; keep the single full AG.
AGQ_BOUNDS = [int(t) for t in
              os.environ.get('GNN_AGQ', '50').split(',')]
NQ = len(AGQ_BOUNDS)
_BQ = [0] + AGQ_BOUNDS
QROWS = [(_BQ[i + 1] - _BQ[i]) * 64 for i in range(NQ)]  # pair rows/rank
QOFF = np.concatenate([[0], np.cumsum([CORES * r for r in QROWS])])[:NQ]


def _pair_row_of(v):
    """Table pair-row index for new-node id v (quarter-major layout)."""
    v = np.asarray(v, np.int64)
    c = v // NPC
    lb = v % NPC
    b = lb // 128
    qi = np.searchsorted(np.asarray(AGQ_BOUNDS), b, side="right")
    qrows = np.asarray(QROWS, np.int64)[qi]
    pq = (lb - np.asarray(_BQ, np.int64)[qi] * 128) // 2
    return QOFF[qi] + c * qrows + pq


# --------------------------------------------------------------------------
# host-side preprocessing
# --------------------------------------------------------------------------

def _wrap_idx(seq):
    """dma_gather idx layout: slot i -> partition i%16, col i//16, the
    16-partition block replicated to all 8 Q7 stripes."""
    seq = np.asarray(seq, np.int16)
    assert len(seq) % 16 == 0
    return np.tile(seq.reshape(-1, 16).T, (8, 1))


def _schedule(cnt_max):
    """Static chunk schedule shared by all cores.

    cnt_max: [NSLOT, 2] max (over cores) edge count per (block, parity).
    Chunks from consecutive blocks (same parity) are merged into calls
    of up to CAP chunks; q0/q1 calls are interleaved in block order so
    each block's two parities become available close together.
    """
    CH = np.maximum(np.ceil(cnt_max / 128).astype(int), 1)  # chunks per (b,q)
    flat = {0: [], 1: []}
    for b in range(NSLOT):
        for q in (0, 1):
            for j in range(int(CH[b, q])):
                flat[q].append((b, j))
    calls_q = {q: [flat[q][i:i + CAP] for i in range(0, len(flat[q]), CAP)]
               for q in (0, 1)}
    calls = []
    block_chunks = [[] for _ in range(NSLOT)]  # (call_idx, local_j, col)
    idxcols = 0
    chtot = 0
    for i in range(max(len(calls_q[0]), len(calls_q[1]))):
        for q in (0, 1):
            if i >= len(calls_q[q]):
                continue
            grp = calls_q[q][i]
            nch = len(grp)
            ci = len(calls)
            calls.append(dict(q=q, col0=idxcols, ch0=chtot, nch=nch,
                              blocks=grp))
            for loc, (b, j) in enumerate(grp):
                block_chunks[b].append((ci, loc, chtot + loc))
            idxcols += nch * 8
            chtot += nch
    return dict(CH=CH, calls=calls, block_chunks=block_chunks,
                idxcols=idxcols, chtot=chtot)


def _prep(inputs):
    x = np.asarray(inputs["x"], np.float32)
    es = np.asarray(inputs["edge_src"]).astype(np.int64)
    ed = np.asarray(inputs["edge_dst"]).astype(np.int64)
    E = es.shape[0]

    v = np.arange(N, dtype=np.int64)
    g = v // NPG
    n = v % NPG
    perm = (n // NSLOT) * NPC + (n % NSLOT) * G + g  # orig id -> new id
    xn = np.empty_like(x)
    xn[perm] = x

    s2 = perm[es]
    d2 = perm[ed]
    s_all = np.concatenate([s2, v])
    d_all = np.concatenate([d2, v])
    deg = np.bincount(d_all, minlength=N).astype(np.float64)
    dinv = (1.0 / np.sqrt(deg)).astype(np.float32)

    c_of = d_all // NPC
    b_of = (d_all % NPC) // 128
    dl_of = (d_all % 128).astype(np.float32)
    pr_of = _pair_row_of(s_all).astype(np.int16)
    q_of = (s_all % 2).astype(np.int64)

    key = (c_of * NSLOT + b_of) * 2 + q_of
    order = np.argsort(key, kind="stable")
    key_sorted = key[order]
    starts = np.searchsorted(key_sorted, np.arange(CORES * NSLOT * 2 + 1))
    cnt = (starts[1:] - starts[:-1]).reshape(CORES, NSLOT, 2)

    sch = _schedule(cnt.max(axis=0))
    CH = sch["CH"]

    # gather-table t=0: dinv * x, padded, quarter-major pair layout
    zt0 = np.zeros((N, DP), np.float16)
    zt0[:, :F] = (dinv[:, None] * xn).astype(np.float16)
    tab0 = np.zeros((PR, 2, DP), np.float16)
    tab0[_pair_row_of(v), v % 2, :] = zt0
    tab0 = np.ascontiguousarray(tab0.reshape(PR, 2 * DP))

    # head weights (shared precompute)
    w1 = np.asarray(inputs["w1"], np.float32)
    bn0_g = np.asarray(inputs["bn0_g"], np.float32)
    bn0_b = np.asarray(inputs["bn0_b"], np.float32)
    b1 = np.asarray(inputs["b1"], np.float32)
    w1g = w1 * bn0_g[:, None]
    b1full = (b1.astype(np.float64)
              + bn0_b.astype(np.float64) @ w1.astype(np.float64)).astype(np.float32)

    def rep(a, cols=None):
        a = np.asarray(a, np.float32).reshape(1, -1)
        return np.ascontiguousarray(np.tile(a, (G, 1)))

    shared = {
        "b1rep": rep(b1full),
        "bn1g": rep(inputs["bn1_g"]), "bn1b": rep(inputs["bn1_b"]),
        "w2": np.asarray(inputs["w2"], np.float32),
        "b2rep": rep(inputs["b2"]),
        "bn2g": rep(inputs["bn2_g"]), "bn2b": rep(inputs["bn2_b"]),
        "w3": np.asarray(inputs["w3"], np.float32),
        "b3rep": rep(inputs["b3"]),
        "bn3g": rep(inputs["bn3_g"]), "bn3b": rep(inputs["bn3_b"]),
        "w4": np.asarray(inputs["w4"], np.float32),
        "b4rep": rep(inputs["b4"]),
        "iota": np.ascontiguousarray(
            np.tile(np.arange(128, dtype=np.float32)[None, :], (128, 1))),
        "ident": np.eye(128, dtype=np.float32),
        "ones": np.ones((128, 128), np.float32),
        "tab0": tab0,
    }

    in_maps = []
    for c in range(CORES):
        idx_arr = np.zeros((128, sch["idxcols"]), np.int16)
        dst_arr = np.full((128, sch["chtot"]), 300.0, np.float32)
        callseq = {ci: np.zeros(call["nch"] * 128, np.int16)
                   for ci, call in enumerate(sch["calls"])}
        for b in range(NSLOT):
            for q in (0, 1):
                kk = (c * NSLOT + b) * 2 + q
                sl = order[starts[kk]:starts[kk + 1]]
                cntE = len(sl)
                chunks = [t for t in sch["block_chunks"][b]
                          if sch["calls"][t[0]]["q"] == q]
                m = len(chunks)
                assert cntE <= m * 128
                seq = np.zeros(m * 128, np.int16)
                seq[:cntE] = pr_of[sl]
                dl = np.full(m * 128, 300.0, np.float32)
                dl[:cntE] = dl_of[sl]
                for j, (ci, loc, col) in enumerate(chunks):
                    callseq[ci][loc * 128:(loc + 1) * 128] = seq[j * 128:(j + 1) * 128]
                    dst_arr[:, col] = dl[j * 128:(j + 1) * 128]
        for ci, call in enumerate(sch["calls"]):
            idx_arr[:, call["col0"]: call["col0"] + call["nch"] * 8] = \
                _wrap_idx(callseq[ci])

        xc = xn[c * NPC:(c + 1) * NPC].reshape(NSLOT, G, F)
        xres = np.ascontiguousarray(xc.transpose(1, 0, 2).reshape(G, NSLOT * F))
        dvc = np.ascontiguousarray(
            dinv[c * NPC:(c + 1) * NPC].reshape(NSLOT, 128).T)

        base = ((NSLOT * c + np.arange(NSLOT))[:, None] * NPG)
        w1p = np.zeros((NSLOT, 4, 128, H1), np.float16)
        for ft in range(4):
            nf = min(128, F - ft * 128)
            rows = base + ft * 128 + np.arange(nf)[None, :]
            w1p[:, ft, :nf, :] = w1g[rows].astype(np.float16)
        w1p = np.ascontiguousarray(w1p.reshape(NSLOT * 4 * 128, H1))

        m = {"idx": idx_arr, "dst": dst_arr, "xres": xres, "dvc": dvc,
             "w1p": w1p}
        m.update(shared)
        in_maps.append(m)

    return in_maps, sch


# --------------------------------------------------------------------------
# device program
# --------------------------------------------------------------------------

def _build(sch):
    nc = bacc.Bacc("TRN2", target_bir_lowering=False, debug=False,
                   num_devices=CORES)

    tab0 = nc.dram_tensor("tab0", [PR, 2 * DP], F16, kind="ExternalInput")
    idxp = nc.dram_tensor("idx", [128, sch["idxcols"]], I16, kind="ExternalInput")
    dstp = nc.dram_tensor("dst", [128, sch["chtot"]], F32, kind="ExternalInput")
    xrep = nc.dram_tensor("xres", [128, NSLOT * F], F32, kind="ExternalInput")
    dvp = nc.dram_tensor("dvc", [128, NSLOT], F32, kind="ExternalInput")
    w1p = nc.dram_tensor("w1p", [NSLOT * 4 * 128, H1], F16, kind="ExternalInput")
    small = {}
    for nm, shp in [("b1rep", [G, H1]), ("bn1g", [G, H1]), ("bn1b", [G, H1]),
                    ("w2", [H1, H2]), ("b2rep", [G, H2]),
                    ("bn2g", [G, H2]), ("bn2b", [G, H2]),
                    ("w3", [H2, H2]), ("b3rep", [G, H2]),
                    ("bn3g", [G, H2]), ("bn3b", [G, H2]),
                    ("w4", [H2, NCLS]), ("b4rep", [G, NCLS]),
                    ("iota", [128, 128]), ("ident", [128, 128]),
                    ("ones", [128, 128])]:
        small[nm] = nc.dram_tensor(nm, shp, F32, kind="ExternalInput")
    outp = nc.dram_tensor("out", [G, NCLS], F32, kind="ExternalOutput")
    # double-buffered AllGather table in pair-shared HBM (fast CC path)
    tabs = [nc.dram_tensor("tab_shA", [PR, 2 * DP], F16, addr_space="Shared"),
            nc.dram_tensor("tab_shB", [PR, 2 * DP], F16, addr_space="Shared")]

    rg = [list(range(CORES))]

    with tile.TileContext(nc) as tc:
        with (
            tc.tile_pool(name="const", bufs=1) as cpool,
            tc.tile_pool(name="dram", bufs=1, space="DRAM") as dpool,
        ):
            nc.gpsimd.load_library(_mlp_lib)

            idx_t = cpool.tile([128, sch["idxcols"]], I16)
            nc.sync.dma_start(out=idx_t[:], in_=idxp[:])
            dst_t = cpool.tile([128, sch["chtot"]], F32)
            nc.sync.dma_start(out=dst_t[:], in_=dstp[:])
            x_t = cpool.tile([128, NSLOT * F], F32)
            nc.sync.dma_start(out=x_t[:], in_=xrep[:])
            dv_t = cpool.tile([128, NSLOT], F32)
            nc.sync.dma_start(out=dv_t[:], in_=dvp[:])
            iota_t = cpool.tile([128, 128], F32)
            nc.sync.dma_start(out=iota_t[:], in_=small["iota"][:])
            ident_t = cpool.tile([128, 128], F32)
            nc.sync.dma_start(out=ident_t[:], in_=small["ident"][:])
            ones_t = cpool.tile([128, 128], F32)
            nc.sync.dma_start(out=ones_t[:], in_=small["ones"][:])
            c_tiny = cpool.tile([128, 1], F32)
            nc.vector.memset(c_tiny[:], 1e-30)
            c_nlam = cpool.tile([128, 1], F32)
            nc.vector.memset(c_nlam[:], -LAM)
            c_eps = cpool.tile([128, 1], F32)
            nc.vector.memset(c_eps[:], BN_EPS)

            sm_t = {}
            for nm in ["b1rep", "bn1g", "bn1b", "w2", "b2rep", "bn2g",
                       "bn2b", "w3", "b3rep", "bn3g", "bn3b", "w4",
                       "b4rep"]:
                shp = list(small[nm].shape)
                t = cpool.tile(shp, F32, tag=nm)
                nc.sync.dma_start(out=t[:], in_=small[nm][:])
                sm_t[nm] = t

            zloc = dpool.tile([NPC, DP], F16)

            with (
                tc.tile_pool(name="gt", bufs=4) as gpool,
                tc.tile_pool(name="sel", bufs=6) as selpool,
                tc.tile_pool(name="wk", bufs=3) as wpool,
                tc.tile_pool(name="sm", bufs=4) as spool,
                tc.tile_pool(name="ps", bufs=2, space="PSUM") as ppool,
                tc.tile_pool(name="hw", bufs=3) as hpool,
                tc.tile_pool(name="hs", bufs=4) as hspool,
                tc.tile_pool(name="hps", bufs=2, space="PSUM") as hppool,
                tc.tile_pool(name="hacc", bufs=1, space="PSUM") as haccp,
            ):
                psw = haccp.tile([G, H1], F32, tag="psw")

                # ---------------- K propagation steps ----------------
                for k in range(K):
                    src = tab0 if k == 0 else tabs[(k - 1) % 2]
                    tab_next = tabs[k % 2]
                    call_tiles = {}
                    for ci, call in enumerate(sch["calls"]):
                        gt = gpool.tile([128, call["nch"], DP], F16, tag="gt")
                        call_tiles[ci] = gt
                        q = call["q"]
                        nidx = call["nch"] * 128
                        nc.gpsimd.dma_gather(
                            out_ap=gt[:],
                            in_ap=src[:, q * DP:(q + 1) * DP],
                            idxs_ap=idx_t[:, call["col0"]:call["col0"] + call["nch"] * 8],
                            num_idxs=nidx,
                            num_idxs_reg=nidx,
                            elem_size=DP,
                            elem_step=2 * DP,
                        )
                    qi = 0  # next AllGather quarter to emit
                    for b in range(NSLOT):
                        chunks = sch["block_chunks"][b]
                        ps = ppool.tile([128, F], F32, tag="ps")
                        for t, (ci, loc, col) in enumerate(chunks):
                            sel = selpool.tile([128, 128], F16, tag="sel")
                            nc.vector.tensor_tensor(
                                out=sel[:],
                                in0=dst_t[:, col:col + 1].to_broadcast([128, 128]),
                                in1=iota_t[:],
                                op=OP.is_equal,
                            )
                            nc.tensor.matmul(
                                out=ps[:],
                                lhsT=sel[:],
                                rhs=call_tiles[ci][:, loc, 0:F],
                                start=(t == 0),
                                stop=(t == len(chunks) - 1),
                            )
                        dcol = dv_t[:, b:b + 1]
                        xblk = x_t[:, b * F:(b + 1) * F]
                        d = wpool.tile([128, F], F32, tag="d")
                        nc.vector.scalar_tensor_tensor(
                            out=d[:], in0=ps[:], scalar=dcol, in1=xblk,
                            op0=OP.mult, op1=OP.subtract)
                        dsq = wpool.tile([128, F], F32, tag="dsq")
                        rn2 = spool.tile([128, 1], F32, tag="rn2")
                        nc.vector.tensor_tensor(out=dsq[:], in0=d[:],
                                                in1=d[:], op=OP.mult)
                        nc.vector.tensor_reduce(
                            out=rn2[:], in_=dsq[:],
                            axis=mybir.AxisListType.X, op=OP.add)
                        rn = spool.tile([128, 1], F32, tag="rn")
                        nc.scalar.activation(rn[:], rn2[:], AF.Sqrt, bias=c_tiny[:])
                        sc = spool.tile([128, 1], F32, tag="sc")
                        nc.scalar.activation(sc[:], rn[:], AF.Relu, bias=c_nlam[:])
                        rr = spool.tile([128, 1], F32, tag="rr")
                        nc.vector.reciprocal(rr[:], rn[:])
                        s = spool.tile([128, 1], F32, tag="s")
                        nc.vector.tensor_tensor(out=s[:], in0=sc[:], in1=rr[:],
                                                op=OP.mult)
                        z = wpool.tile([128, F], F32, tag="z")
                        nc.vector.scalar_tensor_tensor(
                            out=z[:], in0=d[:], scalar=s[:], in1=xblk,
                            op0=OP.mult, op1=OP.add)
                        if k < K - 1:
                            zt = wpool.tile([128, F], F16, tag="zt")
                            nc.scalar.activation(zt[:], z[:], AF.Identity,
                                                 scale=dcol)
                            nc.sync.dma_start(
                                out=zloc[128 * b:128 * (b + 1), 0:F],
                                in_=zt[:])
                            # emit AllGather quarter once its blocks are done
                            if b + 1 == AGQ_BOUNDS[qi]:
                                b0 = AGQ_BOUNDS[qi - 1] if qi else 0
                                b1 = AGQ_BOUNDS[qi]
                                in_ap = zloc[128 * b0:128 * b1, :].opt()
                                r0 = int(QOFF[qi])
                                out_ap = tab_next[
                                    r0:r0 + CORES * QROWS[qi], :].opt()
                                nc.gpsimd.collective_compute(
                                    "AllGather", OP.bypass, replica_groups=rg,
                                    ins=[in_ap], outs=[out_ap])
                                qi += 1
                        else:
                            # ---- head fused into the last step ----
                            for ft in range(4):
                                fl = min(128, F - ft * 128)
                                tp = hppool.tile([128, 128], F32, tag="tp")
                                hT = hpool.tile([128, G], F32, tag="hT")
                                if fl < 128:
                                    nc.vector.memset(hT[:], 0.0)
                                nc.tensor.transpose(
                                    out=tp[:fl, :],
                                    in_=z[:, ft * 128:ft * 128 + fl],
                                    identity=ident_t[:])
                                nc.scalar.copy(hT[:fl, :], tp[:fl, :])
                                smv = hspool.tile([128, 1], F32, tag="smv")
                                nc.vector.tensor_reduce(
                                    out=smv[:], in_=hT[:],
                                    axis=mybir.AxisListType.X, op=OP.add)
                                sq = hpool.tile([128, G], F32, tag="sq")
                                s2t = hspool.tile([128, 1], F32, tag="s2t")
                                nc.scalar.activation(sq[:], hT[:], AF.Square,
                                                     accum_out=s2t[:])
                                mu = hspool.tile([128, 1], F32, tag="mu")
                                nc.vector.tensor_scalar_mul(mu[:], smv[:], 1.0 / G)
                                musq = hspool.tile([128, 1], F32, tag="musq")
                                nc.vector.tensor_tensor(out=musq[:], in0=mu[:],
                                                        in1=mu[:], op=OP.mult)
                                var = hspool.tile([128, 1], F32, tag="var")
                                nc.vector.scalar_tensor_tensor(
                                    out=var[:], in0=s2t[:], scalar=1.0 / G,
                                    in1=musq[:], op0=OP.mult, op1=OP.subtract)
                                sd = hspool.tile([128, 1], F32, tag="sd")
                                nc.scalar.activation(sd[:], var[:], AF.Sqrt,
                                                     bias=c_eps[:])
                                ri = hspool.tile([128, 1], F32, tag="ri")
                                nc.vector.reciprocal(ri[:], sd[:])
                                nb = hspool.tile([128, 1], F32, tag="nb")
                                nc.vector.scalar_tensor_tensor(
                                    out=nb[:], in0=mu[:], scalar=-1.0, in1=ri[:],
                                    op0=OP.mult, op1=OP.mult)
                                hbn = hpool.tile([128, G], F16, tag="hbn")
                                nc.scalar.activation(hbn[:], hT[:], AF.Identity,
                                                     bias=nb[:], scale=ri[:])
                                w1t = hpool.tile([128, H1], F16, tag="w1t")
                                nc.sync.dma_start(
                                    out=w1t[:],
                                    in_=w1p[(b * 4 + ft) * 128:
                                            (b * 4 + ft + 1) * 128, :])
                                nc.tensor.matmul(
                                    out=psw[:], lhsT=hbn[:], rhs=w1t[:],
                                    start=(b == 0 and ft == 0),
                                    stop=(b == NSLOT - 1 and ft == 3))

                # ---------------- head tail ----------------
                h1p = hpool.tile([G, H1], F32, tag="h1p")
                nc.vector.tensor_copy(out=h1p[:], in_=psw[:])
                ar_in = dpool.tile([G, H1], F32)
                ar_out = dpool.tile([G, H1], F32)
                nc.sync.dma_start(out=ar_in[:], in_=h1p[:])
                nc.gpsimd.collective_compute(
                    "AllReduce", OP.add, replica_groups=rg,
                    ins=[ar_in.opt()], outs=[ar_out.opt()])
                h1 = hpool.tile([G, H1], F32, tag="h1")
                nc.sync.dma_start(out=h1[:], in_=ar_out[:])
                nc.vector.tensor_tensor(out=h1[:], in0=h1[:],
                                        in1=sm_t["b1rep"][:], op=OP.add)

                def bn_over_graphs(h, width, gam, bet):
                    """BatchNorm over the graph axis (partitions) of
                    h [G, width]; returns normalized SBUF tile."""
                    pss = hppool.tile([G, width], F32, tag="st")
                    nc.tensor.matmul(out=pss[:], lhsT=ones_t[:],
                                     rhs=h[:], start=True, stop=True)
                    sqh = hpool.tile([G, width], F32, tag="sqh")
                    nc.scalar.activation(sqh[:], h[:], AF.Square)
                    psq = hppool.tile([G, width], F32, tag="st")
                    nc.tensor.matmul(out=psq[:], lhsT=ones_t[:],
                                     rhs=sqh[:], start=True, stop=True)
                    muh = hpool.tile([G, width], F32, tag="muh")
                    nc.vector.tensor_scalar_mul(muh[:], pss[:], 1.0 / G)
                    m2 = hpool.tile([G, width], F32, tag="m2")
                    nc.vector.tensor_tensor(out=m2[:], in0=muh[:],
                                            in1=muh[:], op=OP.mult)
                    varh = hpool.tile([G, width], F32, tag="varh")
                    nc.vector.scalar_tensor_tensor(
                        out=varh[:], in0=psq[:], scalar=1.0 / G, in1=m2[:],
                        op0=OP.mult, op1=OP.subtract)
                    sdh = hpool.tile([G, width], F32, tag="sdh")
                    nc.scalar.activation(sdh[:], varh[:], AF.Sqrt, bias=c_eps[:])
                    rih = hpool.tile([G, width], F32, tag="rih")
                    nc.vector.reciprocal(rih[:], sdh[:])
                    cen = hpool.tile([G, width], F32, tag="cen")
                    nc.vector.tensor_tensor(out=cen[:], in0=h[:], in1=muh[:],
                                            op=OP.subtract)
                    nrm = hpool.tile([G, width], F32, tag="nrm")
                    nc.vector.tensor_tensor(out=nrm[:], in0=cen[:],
                                            in1=rih[:], op=OP.mult)
                    scl = hpool.tile([G, width], F32, tag="scl")
                    nc.vector.tensor_tensor(out=scl[:], in0=nrm[:],
                                            in1=gam[:, 0:width], op=OP.mult)
                    shf = hpool.tile([G, width], F32, tag="shf")
                    nc.vector.tensor_tensor(out=shf[:], in0=scl[:],
                                            in1=bet[:, 0:width], op=OP.add)
                    return shf

                hb1 = bn_over_graphs(h1, H1, sm_t["bn1g"], sm_t["bn1b"])
                h1r = hpool.tile([G, H1], F32, tag="h1r")
                nc.scalar.activation(h1r[:], hb1[:], AF.Relu)

                def linear(hin, win, wout, wt, brep):
                    tpl = hppool.tile([128, 128], F32, tag="tp")
                    nc.tensor.transpose(out=tpl[:win, :], in_=hin[:, 0:win],
                                        identity=ident_t[:])
                    hTl = hpool.tile([128, G], F32, tag="hTl")
                    nc.scalar.copy(hTl[:win, :], tpl[:win, :])
                    pso = hppool.tile([G, wout], F32, tag="st")
                    nc.tensor.matmul(out=pso[:], lhsT=hTl[:win, :],
                                     rhs=wt[:, 0:wout], start=True, stop=True)
                    ho = hpool.tile([G, wout], F32, tag="ho")
                    nc.vector.tensor_tensor(out=ho[:], in0=pso[:],
                                            in1=brep[:, 0:wout], op=OP.add)
                    return ho

                h2 = linear(h1r, H1, H2, sm_t["w2"], sm_t["b2rep"])
                hb2 = bn_over_graphs(h2, H2, sm_t["bn2g"], sm_t["bn2b"])
                h2r = hpool.tile([G, H2], F32, tag="h2r")
                nc.scalar.activation(h2r[:], hb2[:], AF.Relu)

                h3 = linear(h2r, H2, H2, sm_t["w3"], sm_t["b3rep"])
                hb3 = bn_over_graphs(h3, H2, sm_t["bn3g"], sm_t["bn3b"])
                h3r = hpool.tile([G, H2], F32, tag="h3r")
                nc.scalar.activation(h3r[:], hb3[:], AF.Relu)

                logits = linear(h3r, H2, NCLS, sm_t["w4"], sm_t["b4rep"])
                mx = hspool.tile([G, 1], F32, tag="mx")
                nc.vector.tensor_reduce(out=mx[:], in_=logits[:],
                                        axis=mybir.AxisListType.X, op=OP.max)
                tshift = hpool.tile([G, NCLS], F32, tag="tshift")
                nc.vector.tensor_scalar_sub(tshift[:], logits[:], mx[:])
                ex = hpool.tile([G, NCLS], F32, tag="ex")
                se = hspool.tile([G, 1], F32, tag="se")
                nc.scalar.activation(ex[:], tshift[:], AF.Exp,
                                     accum_out=se[:])
                lse = hspool.tile([G, 1], F32, tag="lse")
                nc.scalar.activation(lse[:], se[:], AF.Ln)
                res = hpool.tile([G, NCLS], F32, tag="res")
                nc.vector.tensor_scalar_sub(res[:], tshift[:], lse[:])
                nc.sync.dma_start(out=outp[:], in_=res[:])

    nc.compile()
    return nc


_CACHE = {}


def kernel(**inputs):
    in_maps, sch = _prep(inputs)
    key = (sch["idxcols"], sch["chtot"],
           tuple(call["nch"] for call in sch["calls"]))
    if key not in _CACHE:
        import time as _t
        _t0 = _t.time()
        _CACHE[key] = _build(sch)
        print(f"[kernel] build+compile {_t.time()-_t0:.1f}s", flush=True)
    nc = _CACHE[key]
    import time as _t
    _t0 = _t.time()
    res = run_bass_kernel_spmd(nc, in_maps, core_ids=list(range(CORES)))
    print(f"[kernel] device run {_t.time()-_t0:.1f}s", flush=True)
    return np.asarray(res.results[0]["out"], np.float32)


# revision 14
# speedup vs baseline: 1.3330x; 1.3330x over previous
"""AirGC GNN message-passing kernel for 8 Trainium2 NeuronCores.

Strategy
--------
Nodes are relabeled so core c owns the nodes that become h-features
[20000c, 20000(c+1)) of the BN-MLP head: orig node v=(g,n) with g=graph,
n=node-in-graph maps to core j=n//50, local row (n%50)*128+g.  Each core
owns 6400 destination rows (50 blocks of 128; partition = graph id).

K=10 propagation steps (gamma*2*(1-lam)==1 so y = A_hat z):
  - z~ = dinv * z is kept as an fp16 DRAM gather table of paired rows
    [25600, 1024] (node n at pair row n//2, column half n%2; 400 feats
    padded to 512 so each half is a 1024B 256-multiple).
  - per dst block of 128 nodes, in-edges (incl. self loops) are fetched
    with dma_gather (int16 pair-row indices, elem_step=1024 fp16).
    Chunks of 128 edges are merged into large calls (CAP chunks each,
    split by source parity, parity-interleaved in block order) to
    amortize the ~1us/call SWDGE fixed cost; Q7 descriptor generation
    (~7.3ns/idx) is the critical-path wall.
  - segment-sum via TensorE: psum[128dst,400] += Sel^T[128e,128d] @ G,
    Sel built on VectorE with is_equal(dst_local, iota).
  - AirGC update: d = dinv_dst*psum - x; rn=|d|; s=relu(rn-.5)/rn;
    z' = x + s*d; table row = dinv*z' (fp16).
  - The table is double-buffered in pair-shared ("Shared") HBM; the
    inter-step exchange is an AllGather split into 4 block-quarters so
    the first 3 quarters overlap the tail of the gather burst and only
    the last quarter's latency is exposed.
Head: fused into step K-1's block loop: transposed blocks put
h-features on partitions so BN0 stats are row reductions; (h-mu)*rinv
contracts with host-side bn0_g-folded w1 (fp16) into [128,128]
partials; one 64KB AllReduce; BN1..3 + MLP tail replicated per core
(stats over graphs via ones-matmul).
"""

import os
import sys

for _p in ("/opt/trn_rl_repo", "/root/.axon_site/_ro/trn_rl_repo"):
    if os.path.isdir(_p) and _p not in sys.path:
        sys.path.insert(0, _p)

import numpy as np

from concourse import bacc, bass, mybir, tile
from concourse.bass_utils import run_bass_kernel_spmd
from concourse.library_config import mlp as _mlp_lib

F16 = mybir.dt.float16
F32 = mybir.dt.float32
I16 = mybir.dt.int16
OP = mybir.AluOpType
AF = mybir.ActivationFunctionType

N = 51200          # nodes
F = 400            # features per node
G = 128            # graphs
NPG = 400          # nodes per graph
CORES = 8
NPC = N // CORES   # 6400 nodes per core
NSLOT = NPG // CORES  # 50 node-slots per core == blocks per core
K = int(os.environ.get('GNN_K', '10'))
CAP = int(os.environ.get('GNN_CAP', '5'))  # >5 chunks/call hangs SWDGE
LAM = 0.5
BN_EPS = 1e-5
DP = 512           # padded feature columns (fp16, 1024B)
PR = N // 2        # 25600 pair rows in gather table
H1, H2, NCLS = 128, 64, 10
# AllGather split boundaries (block index); table is quarter-major so each
# AG quarter writes one contiguous slice:
# row = QOFF[q] + core*QROWS[q] + local-pair-within-quarter
# NOTE: splitting the AG (e.g. '13,26,38,50') deadlocks on device —
# offset-output collectives are not safe here; keep the single full AG.
AGQ_BOUNDS = [int(t) for t in
              os.environ.get('GNN_AGQ', '50').split(',')]
NQ = len(AGQ_BOUNDS)
_BQ = [0] + AGQ_BOUNDS
QROWS = [(_BQ[i + 1] - _BQ[i]) * 64 for i in range(NQ)]  # pair rows/rank
QOFF = np.concatenate([[0], np.cumsum([CORES * r for r in QROWS])])[:NQ]


def _pair_row_of(v):
    """Table pair-row index for new-node id v (quarter-major layout)."""
    v = np.asarray(v, np.int64)
    c = v // NPC
    lb = v % NPC
    b = lb // 128
    qi = np.searchsorted(np.asarray(AGQ_BOUNDS), b, side="right")
    qrows = np.asarray(QROWS, np.int64)[qi]
    pq = (lb - np.asarray(_BQ, np.int64)[qi] * 128) // 2
    return QOFF[qi] + c * qrows + pq


# --------------------------------------------------------------------------
# host-side preprocessing
# --------------------------------------------------------------------------

def _wrap_idx(seq):
    """dma_gather idx layout: slot i -> partition i%16, col i//16, the
    16-partition block replicated to all 8 Q7 stripes."""
    seq = np.asarray(seq, np.int16)
    assert len(seq) % 16 == 0
    return np.tile(seq.reshape(-1, 16).T, (8, 1))


def _schedule(cnt_max):
    """Static chunk schedule shared by all cores.

    cnt_max: [NSLOT, 2] max (over cores) edge count per (block, parity).
    Chunks from consecutive blocks (same parity) are merged into calls
    of up to CAP chunks; q0/q1 calls are interleaved in block order so
    each block's two parities become available close together.
    """
    CH = np.maximum(np.ceil(cnt_max / 128).astype(int), 1)  # chunks per (b,q)
    flat = {0: [], 1: []}
    for b in range(NSLOT):
        for q in (0, 1):
            for j in range(int(CH[b, q])):
                flat[q].append((b, j))
    calls_q = {q: [flat[q][i:i + CAP] for i in range(0, len(flat[q]), CAP)]
               for q in (0, 1)}
    calls = []
    block_chunks = [[] for _ in range(NSLOT)]  # (call_idx, local_j, col)
    idxcols = 0
    chtot = 0
    for i in range(max(len(calls_q[0]), len(calls_q[1]))):
        for q in (0, 1):
            if i >= len(calls_q[q]):
                continue
            grp = calls_q[q][i]
            nch = len(grp)
            ci = len(calls)
            calls.append(dict(q=q, col0=idxcols, ch0=chtot, nch=nch,
                              blocks=grp))
            for loc, (b, j) in enumerate(grp):
                block_chunks[b].append((ci, loc, chtot + loc))
            idxcols += nch * 8
            chtot += nch
    return dict(CH=CH, calls=calls, block_chunks=block_chunks,
                idxcols=idxcols, chtot=chtot)


def _prep(inputs):
    x = np.asarray(inputs["x"], np.float32)
    es = np.asarray(inputs["edge_src"]).astype(np.int64)
    ed = np.asarray(inputs["edge_dst"]).astype(np.int64)
    E = es.shape[0]

    v = np.arange(N, dtype=np.int64)
    g = v // NPG
    n = v % NPG
    perm = (n // NSLOT) * NPC + (n % NSLOT) * G + g  # orig id -> new id
    xn = np.empty_like(x)
    xn[perm] = x

    s2 = perm[es]
    d2 = perm[ed]
    s_all = np.concatenate([s2, v])
    d_all = np.concatenate([d2, v])
    deg = np.bincount(d_all, minlength=N).astype(np.float64)
    dinv = (1.0 / np.sqrt(deg)).astype(np.float32)

    c_of = d_all // NPC
    b_of = (d_all % NPC) // 128
    dl_of = (d_all % 128).astype(np.float32)
    pr_of = _pair_row_of(s_all).astype(np.int16)
    q_of = (s_all % 2).astype(np.int64)

    key = (c_of * NSLOT + b_of) * 2 + q_of
    order = np.argsort(key, kind="stable")
    key_sorted = key[order]
    starts = np.searchsorted(key_sorted, np.arange(CORES * NSLOT * 2 + 1))
    cnt = (starts[1:] - starts[:-1]).reshape(CORES, NSLOT, 2)

    sch = _schedule(cnt.max(axis=0))
    CH = sch["CH"]

    # gather-table t=0: dinv * x, padded, quarter-major pair layout
    zt0 = np.zeros((N, DP), np.float16)
    zt0[:, :F] = (dinv[:, None] * xn).astype(np.float16)
    tab0 = np.zeros((PR, 2, DP), np.float16)
    tab0[_pair_row_of(v), v % 2, :] = zt0
    tab0 = np.ascontiguousarray(tab0.reshape(PR, 2 * DP))

    # head weights (shared precompute)
    w1 = np.asarray(inputs["w1"], np.float32)
    bn0_g = np.asarray(inputs["bn0_g"], np.float32)
    bn0_b = np.asarray(inputs["bn0_b"], np.float32)
    b1 = np.asarray(inputs["b1"], np.float32)
    w1g = w1 * bn0_g[:, None]
    b1full = (b1.astype(np.float64)
              + bn0_b.astype(np.float64) @ w1.astype(np.float64)).astype(np.float32)

    def rep(a, cols=None):
        a = np.asarray(a, np.float32).reshape(1, -1)
        return np.ascontiguousarray(np.tile(a, (G, 1)))

    shared = {
        "b1rep": rep(b1full),
        "bn1g": rep(inputs["bn1_g"]), "bn1b": rep(inputs["bn1_b"]),
        "w2": np.asarray(inputs["w2"], np.float32),
        "b2rep": rep(inputs["b2"]),
        "bn2g": rep(inputs["bn2_g"]), "bn2b": rep(inputs["bn2_b"]),
        "w3": np.asarray(inputs["w3"], np.float32),
        "b3rep": rep(inputs["b3"]),
        "bn3g": rep(inputs["bn3_g"]), "bn3b": rep(inputs["bn3_b"]),
        "w4": np.asarray(inputs["w4"], np.float32),
        "b4rep": rep(inputs["b4"]),
        "iota": np.ascontiguousarray(
            np.tile(np.arange(128, dtype=np.float32)[None, :], (128, 1))),
        "ident": np.eye(128, dtype=np.float32),
        "ones": np.ones((128, 128), np.float32),
        "tab0": tab0,
    }

    in_maps = []
    for c in range(CORES):
        idx_arr = np.zeros((128, sch["idxcols"]), np.int16)
        dst_arr = np.full((128, sch["chtot"]), 300.0, np.float32)
        callseq = {ci: np.zeros(call["nch"] * 128, np.int16)
                   for ci, call in enumerate(sch["calls"])}
        for b in range(NSLOT):
            for q in (0, 1):
                kk = (c * NSLOT + b) * 2 + q
                sl = order[starts[kk]:starts[kk + 1]]
                cntE = len(sl)
                chunks = [t for t in sch["block_chunks"][b]
                          if sch["calls"][t[0]]["q"] == q]
                m = len(chunks)
                assert cntE <= m * 128
                seq = np.zeros(m * 128, np.int16)
                seq[:cntE] = pr_of[sl]
                dl = np.full(m * 128, 300.0, np.float32)
                dl[:cntE] = dl_of[sl]
                for j, (ci, loc, col) in enumerate(chunks):
                    callseq[ci][loc * 128:(loc + 1) * 128] = seq[j * 128:(j + 1) * 128]
                    dst_arr[:, col] = dl[j * 128:(j + 1) * 128]
        for ci, call in enumerate(sch["calls"]):
            idx_arr[:, call["col0"]: call["col0"] + call["nch"] * 8] = \
                _wrap_idx(callseq[ci])

        xc = xn[c * NPC:(c + 1) * NPC].reshape(NSLOT, G, F)
        xres = np.ascontiguousarray(xc.transpose(1, 0, 2).reshape(G, NSLOT * F))
        dvc = np.ascontiguousarray(
            dinv[c * NPC:(c + 1) * NPC].reshape(NSLOT, 128).T)

        base = ((NSLOT * c + np.arange(NSLOT))[:, None] * NPG)
        w1p = np.zeros((NSLOT, 4, 128, H1), np.float16)
        for ft in range(4):
            nf = min(128, F - ft * 128)
            rows = base + ft * 128 + np.arange(nf)[None, :]
            w1p[:, ft, :nf, :] = w1g[rows].astype(np.float16)
        w1p = np.ascontiguousarray(w1p.reshape(NSLOT * 4 * 128, H1))

        m = {"idx": idx_arr, "dst": dst_arr, "xres": xres, "dvc": dvc,
             "w1p": w1p}
        m.update(shared)
        in_maps.append(m)

    return in_maps, sch


# --------------------------------------------------------------------------
# device program
# --------------------------------------------------------------------------

NSWQ = int(os.environ.get('GNN_NSWQ', '1'))  # SWDGE queues (Q7 core pairs)


def _build(sch):
    nc = bacc.Bacc("TRN2", target_bir_lowering=False, debug=False,
                   num_devices=CORES, num_swdge_queues=NSWQ)

    tab0 = nc.dram_tensor("tab0", [PR, 2 * DP], F16, kind="ExternalInput")
    idxp = nc.dram_tensor("idx", [128, sch["idxcols"]], I16, kind="ExternalInput")
    dstp = nc.dram_tensor("dst", [128, sch["chtot"]], F32, kind="ExternalInput")
    xrep = nc.dram_tensor("xres", [128, NSLOT * F], F32, kind="ExternalInput")
    dvp = nc.dram_tensor("dvc", [128, NSLOT], F32, kind="ExternalInput")
    w1p = nc.dram_tensor("w1p", [NSLOT * 4 * 128, H1], F16, kind="ExternalInput")
    small = {}
    for nm, shp in [("b1rep", [G, H1]), ("bn1g", [G, H1]), ("bn1b", [G, H1]),
                    ("w2", [H1, H2]), ("b2rep", [G, H2]),
                    ("bn2g", [G, H2]), ("bn2b", [G, H2]),
                    ("w3", [H2, H2]), ("b3rep", [G, H2]),
                    ("bn3g", [G, H2]), ("bn3b", [G, H2]),
                    ("w4", [H2, NCLS]), ("b4rep", [G, NCLS]),
                    ("iota", [128, 128]), ("ident", [128, 128]),
                    ("ones", [128, 128])]:
        small[nm] = nc.dram_tensor(nm, shp, F32, kind="ExternalInput")
    outp = nc.dram_tensor("out", [G, NCLS], F32, kind="ExternalOutput")
    # double-buffered AllGather table in pair-shared HBM (fast CC path)
    tabs = [nc.dram_tensor("tab_shA", [PR, 2 * DP], F16, addr_space="Shared"),
            nc.dram_tensor("tab_shB", [PR, 2 * DP], F16, addr_space="Shared")]

    rg = [list(range(CORES))]

    with tile.TileContext(nc) as tc:
        with (
            tc.tile_pool(name="const", bufs=1) as cpool,
            tc.tile_pool(name="dram", bufs=1, space="DRAM") as dpool,
        ):
            nc.gpsimd.load_library(_mlp_lib)

            idx_t = cpool.tile([128, sch["idxcols"]], I16)
            nc.sync.dma_start(out=idx_t[:], in_=idxp[:])
            dst_t = cpool.tile([128, sch["chtot"]], F32)
            nc.sync.dma_start(out=dst_t[:], in_=dstp[:])
            x_t = cpool.tile([128, NSLOT * F], F32)
            nc.sync.dma_start(out=x_t[:], in_=xrep[:])
            dv_t = cpool.tile([128, NSLOT], F32)
            nc.sync.dma_start(out=dv_t[:], in_=dvp[:])
            iota_t = cpool.tile([128, 128], F32)
            nc.sync.dma_start(out=iota_t[:], in_=small["iota"][:])
            ident_t = cpool.tile([128, 128], F32)
            nc.sync.dma_start(out=ident_t[:], in_=small["ident"][:])
            ones_t = cpool.tile([128, 128], F32)
            nc.sync.dma_start(out=ones_t[:], in_=small["ones"][:])
            c_tiny = cpool.tile([128, 1], F32)
            nc.vector.memset(c_tiny[:], 1e-30)
            c_nlam = cpool.tile([128, 1], F32)
            nc.vector.memset(c_nlam[:], -LAM)
            c_eps = cpool.tile([128, 1], F32)
            nc.vector.memset(c_eps[:], BN_EPS)

            sm_t = {}
            for nm in ["b1rep", "bn1g", "bn1b", "w2", "b2rep", "bn2g",
                       "bn2b", "w3", "b3rep", "bn3g", "bn3b", "w4",
                       "b4rep"]:
                shp = list(small[nm].shape)
                t = cpool.tile(shp, F32, tag=nm)
                nc.sync.dma_start(out=t[:], in_=small[nm][:])
                sm_t[nm] = t

            zloc = dpool.tile([NPC, DP], F16)

            with (
                tc.tile_pool(name="gt", bufs=4) as gpool,
                tc.tile_pool(name="sel", bufs=6) as selpool,
                tc.tile_pool(name="wk", bufs=3) as wpool,
                tc.tile_pool(name="sm", bufs=4) as spool,
                tc.tile_pool(name="ps", bufs=2, space="PSUM") as ppool,
                tc.tile_pool(name="hw", bufs=3) as hpool,
                tc.tile_pool(name="hs", bufs=4) as hspool,
                tc.tile_pool(name="hps", bufs=2, space="PSUM") as hppool,
                tc.tile_pool(name="hacc", bufs=1, space="PSUM") as haccp,
            ):
                psw = haccp.tile([G, H1], F32, tag="psw")

                # ---------------- K propagation steps ----------------
                for k in range(K):
                    src = tab0 if k == 0 else tabs[(k - 1) % 2]
                    tab_next = tabs[k % 2]
                    call_tiles = {}
                    for ci, call in enumerate(sch["calls"]):
                        gt = gpool.tile([128, call["nch"], DP], F16, tag="gt")
                        call_tiles[ci] = gt
                        q = call["q"]
                        nidx = call["nch"] * 128
                        nc.gpsimd.dma_gather(
                            out_ap=gt[:],
                            in_ap=src[:, q * DP:(q + 1) * DP],
                            idxs_ap=idx_t[:, call["col0"]:call["col0"] + call["nch"] * 8],
                            num_idxs=nidx,
                            num_idxs_reg=nidx,
                            elem_size=DP,
                            elem_step=2 * DP,
                            queue_num=ci % NSWQ,
                        )
                    qi = 0  # next AllGather quarter to emit
                    for b in range(NSLOT):
                        chunks = sch["block_chunks"][b]
                        ps = ppool.tile([128, F], F32, tag="ps")
                        for t, (ci, loc, col) in enumerate(chunks):
                            sel = selpool.tile([128, 128], F16, tag="sel")
                            nc.vector.tensor_tensor(
                                out=sel[:],
                                in0=dst_t[:, col:col + 1].to_broadcast([128, 128]),
                                in1=iota_t[:],
                                op=OP.is_equal,
                            )
                            nc.tensor.matmul(
                                out=ps[:],
                                lhsT=sel[:],
                                rhs=call_tiles[ci][:, loc, 0:F],
                                start=(t == 0),
                                stop=(t == len(chunks) - 1),
                            )
                        dcol = dv_t[:, b:b + 1]
                        xblk = x_t[:, b * F:(b + 1) * F]
                        d = wpool.tile([128, F], F32, tag="d")
                        nc.vector.scalar_tensor_tensor(
                            out=d[:], in0=ps[:], scalar=dcol, in1=xblk,
                            op0=OP.mult, op1=OP.subtract)
                        dsq = wpool.tile([128, F], F32, tag="dsq")
                        rn2 = spool.tile([128, 1], F32, tag="rn2")
                        nc.vector.tensor_tensor(out=dsq[:], in0=d[:],
                                                in1=d[:], op=OP.mult)
                        nc.vector.tensor_reduce(
                            out=rn2[:], in_=dsq[:],
                            axis=mybir.AxisListType.X, op=OP.add)
                        rn = spool.tile([128, 1], F32, tag="rn")
                        nc.scalar.activation(rn[:], rn2[:], AF.Sqrt, bias=c_tiny[:])
                        sc = spool.tile([128, 1], F32, tag="sc")
                        nc.scalar.activation(sc[:], rn[:], AF.Relu, bias=c_nlam[:])
                        rr = spool.tile([128, 1], F32, tag="rr")
                        nc.vector.reciprocal(rr[:], rn[:])
                        s = spool.tile([128, 1], F32, tag="s")
                        nc.vector.tensor_tensor(out=s[:], in0=sc[:], in1=rr[:],
                                                op=OP.mult)
                        z = wpool.tile([128, F], F32, tag="z")
                        nc.vector.scalar_tensor_tensor(
                            out=z[:], in0=d[:], scalar=s[:], in1=xblk,
                            op0=OP.mult, op1=OP.add)
                        if k < K - 1:
                            zt = wpool.tile([128, F], F16, tag="zt")
                            nc.scalar.activation(zt[:], z[:], AF.Identity,
                                                 scale=dcol)
                            nc.sync.dma_start(
                                out=zloc[128 * b:128 * (b + 1), 0:F],
                                in_=zt[:])
                            # emit AllGather quarter once its blocks are done
                            if b + 1 == AGQ_BOUNDS[qi]:
                                b0 = AGQ_BOUNDS[qi - 1] if qi else 0
                                b1 = AGQ_BOUNDS[qi]
                                in_ap = zloc[128 * b0:128 * b1, :].opt()
                                r0 = int(QOFF[qi])
                                out_ap = tab_next[
                                    r0:r0 + CORES * QROWS[qi], :].opt()
                                nc.gpsimd.collective_compute(
                                    "AllGather", OP.bypass, replica_groups=rg,
                                    ins=[in_ap], outs=[out_ap])
                                qi += 1
                        else:
                            # ---- head fused into the last step ----
                            for ft in range(4):
                                fl = min(128, F - ft * 128)
                                tp = hppool.tile([128, 128], F32, tag="tp")
                                hT = hpool.tile([128, G], F32, tag="hT")
                                if fl < 128:
                                    nc.vector.memset(hT[:], 0.0)
                                nc.tensor.transpose(
                                    out=tp[:fl, :],
                                    in_=z[:, ft * 128:ft * 128 + fl],
                                    identity=ident_t[:])
                                nc.scalar.copy(hT[:fl, :], tp[:fl, :])
                                smv = hspool.tile([128, 1], F32, tag="smv")
                                nc.vector.tensor_reduce(
                                    out=smv[:], in_=hT[:],
                                    axis=mybir.AxisListType.X, op=OP.add)
                                sq = hpool.tile([128, G], F32, tag="sq")
                                s2t = hspool.tile([128, 1], F32, tag="s2t")
                                nc.scalar.activation(sq[:], hT[:], AF.Square,
                                                     accum_out=s2t[:])
                                mu = hspool.tile([128, 1], F32, tag="mu")
                                nc.vector.tensor_scalar_mul(mu[:], smv[:], 1.0 / G)
                                musq = hspool.tile([128, 1], F32, tag="musq")
                                nc.vector.tensor_tensor(out=musq[:], in0=mu[:],
                                                        in1=mu[:], op=OP.mult)
                                var = hspool.tile([128, 1], F32, tag="var")
                                nc.vector.scalar_tensor_tensor(
                                    out=var[:], in0=s2t[:], scalar=1.0 / G,
                                    in1=musq[:], op0=OP.mult, op1=OP.subtract)
                                sd = hspool.tile([128, 1], F32, tag="sd")
                                nc.scalar.activation(sd[:], var[:], AF.Sqrt,
                                                     bias=c_eps[:])
                                ri = hspool.tile([128, 1], F32, tag="ri")
                                nc.vector.reciprocal(ri[:], sd[:])
                                nb = hspool.tile([128, 1], F32, tag="nb")
                                nc.vector.scalar_tensor_tensor(
                                    out=nb[:], in0=mu[:], scalar=-1.0, in1=ri[:],
                                    op0=OP.mult, op1=OP.mult)
                                hbn = hpool.tile([128, G], F16, tag="hbn")
                                nc.scalar.activation(hbn[:], hT[:], AF.Identity,
                                                     bias=nb[:], scale=ri[:])
                                w1t = hpool.tile([128, H1], F16, tag="w1t")
                                nc.sync.dma_start(
                                    out=w1t[:],
                                    in_=w1p[(b * 4 + ft) * 128:
                                            (b * 4 + ft + 1) * 128, :])
                                nc.tensor.matmul(
                                    out=psw[:], lhsT=hbn[:], rhs=w1t[:],
                                    start=(b == 0 and ft == 0),
                                    stop=(b == NSLOT - 1 and ft == 3))

                # ---------------- head tail ----------------
                h1p = hpool.tile([G, H1], F32, tag="h1p")
                nc.vector.tensor_copy(out=h1p[:], in_=psw[:])
                ar_in = dpool.tile([G, H1], F32)
                ar_out = dpool.tile([G, H1], F32)
                nc.sync.dma_start(out=ar_in[:], in_=h1p[:])
                nc.gpsimd.collective_compute(
                    "AllReduce", OP.add, replica_groups=rg,
                    ins=[ar_in.opt()], outs=[ar_out.opt()])
                h1 = hpool.tile([G, H1], F32, tag="h1")
                nc.sync.dma_start(out=h1[:], in_=ar_out[:])
                nc.vector.tensor_tensor(out=h1[:], in0=h1[:],
                                        in1=sm_t["b1rep"][:], op=OP.add)

                def bn_over_graphs(h, width, gam, bet):
                    """BatchNorm over the graph axis (partitions) of
                    h [G, width]; returns normalized SBUF tile."""
                    pss = hppool.tile([G, width], F32, tag="st")
                    nc.tensor.matmul(out=pss[:], lhsT=ones_t[:],
                                     rhs=h[:], start=True, stop=True)
                    sqh = hpool.tile([G, width], F32, tag="sqh")
                    nc.scalar.activation(sqh[:], h[:], AF.Square)
                    psq = hppool.tile([G, width], F32, tag="st")
                    nc.tensor.matmul(out=psq[:], lhsT=ones_t[:],
                                     rhs=sqh[:], start=True, stop=True)
                    muh = hpool.tile([G, width], F32, tag="muh")
                    nc.vector.tensor_scalar_mul(muh[:], pss[:], 1.0 / G)
                    m2 = hpool.tile([G, width], F32, tag="m2")
                    nc.vector.tensor_tensor(out=m2[:], in0=muh[:],
                                            in1=muh[:], op=OP.mult)
                    varh = hpool.tile([G, width], F32, tag="varh")
                    nc.vector.scalar_tensor_tensor(
                        out=varh[:], in0=psq[:], scalar=1.0 / G, in1=m2[:],
                        op0=OP.mult, op1=OP.subtract)
                    sdh = hpool.tile([G, width], F32, tag="sdh")
                    nc.scalar.activation(sdh[:], varh[:], AF.Sqrt, bias=c_eps[:])
                    rih = hpool.tile([G, width], F32, tag="rih")
                    nc.vector.reciprocal(rih[:], sdh[:])
                    cen = hpool.tile([G, width], F32, tag="cen")
                    nc.vector.tensor_tensor(out=cen[:], in0=h[:], in1=muh[:],
                                            op=OP.subtract)
                    nrm = hpool.tile([G, width], F32, tag="nrm")
                    nc.vector.tensor_tensor(out=nrm[:], in0=cen[:],
                                            in1=rih[:], op=OP.mult)
                    scl = hpool.tile([G, width], F32, tag="scl")
                    nc.vector.tensor_tensor(out=scl[:], in0=nrm[:],
                                            in1=gam[:, 0:width], op=OP.mult)
                    shf = hpool.tile([G, width], F32, tag="shf")
                    nc.vector.tensor_tensor(out=shf[:], in0=scl[:],
                                            in1=bet[:, 0:width], op=OP.add)
                    return shf

                hb1 = bn_over_graphs(h1, H1, sm_t["bn1g"], sm_t["bn1b"])
                h1r = hpool.tile([G, H1], F32, tag="h1r")
                nc.scalar.activation(h1r[:], hb1[:], AF.Relu)

                def linear(hin, win, wout, wt, brep):
                    tpl = hppool.tile([128, 128], F32, tag="tp")
                    nc.tensor.transpose(out=tpl[:win, :], in_=hin[:, 0:win],
                                        identity=ident_t[:])
                    hTl = hpool.tile([128, G], F32, tag="hTl")
                    nc.scalar.copy(hTl[:win, :], tpl[:win, :])
                    pso = hppool.tile([G, wout], F32, tag="st")
                    nc.tensor.matmul(out=pso[:], lhsT=hTl[:win, :],
                                     rhs=wt[:, 0:wout], start=True, stop=True)
                    ho = hpool.tile([G, wout], F32, tag="ho")
                    nc.vector.tensor_tensor(out=ho[:], in0=pso[:],
                                            in1=brep[:, 0:wout], op=OP.add)
                    return ho

                h2 = linear(h1r, H1, H2, sm_t["w2"], sm_t["b2rep"])
                hb2 = bn_over_graphs(h2, H2, sm_t["bn2g"], sm_t["bn2b"])
                h2r = hpool.tile([G, H2], F32, tag="h2r")
                nc.scalar.activation(h2r[:], hb2[:], AF.Relu)

                h3 = linear(h2r, H2, H2, sm_t["w3"], sm_t["b3rep"])
                hb3 = bn_over_graphs(h3, H2, sm_t["bn3g"], sm_t["bn3b"])
                h3r = hpool.tile([G, H2], F32, tag="h3r")
                nc.scalar.activation(h3r[:], hb3[:], AF.Relu)

                logits = linear(h3r, H2, NCLS, sm_t["w4"], sm_t["b4rep"])
                mx = hspool.tile([G, 1], F32, tag="mx")
                nc.vector.tensor_reduce(out=mx[:], in_=logits[:],
                                        axis=mybir.AxisListType.X, op=OP.max)
                tshift = hpool.tile([G, NCLS], F32, tag="tshift")
                nc.vector.tensor_scalar_sub(tshift[:], logits[:], mx[:])
                ex = hpool.tile([G, NCLS], F32, tag="ex")
                se = hspool.tile([G, 1], F32, tag="se")
                nc.scalar.activation(ex[:], tshift[:], AF.Exp,
                                     accum_out=se[:])
                lse = hspool.tile([G, 1], F32, tag="lse")
                nc.scalar.activation(lse[:], se[:], AF.Ln)
                res = hpool.tile([G, NCLS], F32, tag="res")
                nc.vector.tensor_scalar_sub(res[:], tshift[:], lse[:])
                nc.sync.dma_start(out=outp[:], in_=res[:])

    nc.compile()
    return nc


_CACHE = {}


def kernel(**inputs):
    in_maps, sch = _prep(inputs)
    key = (sch["idxcols"], sch["chtot"],
           tuple(call["nch"] for call in sch["calls"]))
    if key not in _CACHE:
        import time as _t
        _t0 = _t.time()
        _CACHE[key] = _build(sch)
        print(f"[kernel] build+compile {_t.time()-_t0:.1f}s", flush=True)
    nc = _CACHE[key]
    import time as _t
    _t0 = _t.time()
    res = run_bass_kernel_spmd(nc, in_maps, core_ids=list(range(CORES)))
    print(f"[kernel] device run {_t.time()-_t0:.1f}s", flush=True)
    return np.asarray(res.results[0]["out"], np.float32)
